# revision 3
# baseline (speedup 1.0000x reference)
"""CP-decomposed conv (1x1 -> depthwise-h -> depthwise-w -> 1x1) on 8 TRN2
NeuronCores, data-parallel over batch (4 images per core).

V3: per-band engine balancing.  Bands of 6 output rows (21/img).  Two band
pipelines, chosen per band to balance PE/ACT/DVE/Pool:

  FOLD band: h-conv folded into the channel matmul (contract 3*C):
     u_f[r,h',w] (PSUM) = sum_{a,c} x[c,h'+a,w] * (f3*f1[a]*f2[0])[c,r]
     ACT copies u_f -> yh (SBUF bf16); w-taps; stage B.
  UNFOLD band: plain 1x1 matmul (contract C):
     u[r,h,w] (PSUM, 8 rows) = sum_c x[c,h,w] * (f3*f1[0]*f2[0])[c,r]
     ACT copies u -> t0 (SBUF bf16); h-taps via DVE TS(4x)+TT(2x):
        yh = t0[h] + (t0[h+1]*rh1) + (t0[h+2]*rh2)
  w-taps (both): z = yh[w] + yh[w+1]*rw1 + yh[w+2]*rw2 via
     TS t2=yh*rw2 (4x), TT z=yh[+0]+t2[+2] (2x, 4B-aligned shift),
     STT z=(yh[+1]*rw1)+z (1x)  [DVE]  -- or TS t1,t2 on DVE + 2 TT on Pool.
  stage B: 4 bf16 matmuls into one 3-bank PSUM tile (bank-crossing OK),
     single wide PSUM->SBUF bf16 copy (ACT or DVE per band), DMA out.
"""

import numpy as np

B, C, H, W = 32, 256, 128, 128
FH, FW = 3, 3
F, R = 256, 128
HP, WP = H - FH + 1, W - FW + 1  # 126, 126
NCORES = 8
BL = B // NCORES  # images per core

BH = 6               # output rows per band
NB = HP // BH        # 21 bands per image, exact
N1 = BH * 128        # 768
NZ = BH * WP         # 756

# per-band flavor cycle: (fold, w_taps_on_pool, b_copy_on_dve)
F_POOL = (True, True, False)
F_BDVE = (True, False, True)
U_D = (False, False, False)
U_POOL = (False, True, False)
FLAVORS = [F_POOL, U_D, F_BDVE, U_POOL, F_POOL,
           U_D, F_BDVE, U_D, F_POOL, U_POOL]

_NC_CACHE = {}


def _build_nc():
    import concourse.bacc as bacc
    import concourse.mybir as mybir
    import concourse.tile as tile

    dt = mybir.dt
    bf16 = dt.bfloat16
    f32 = dt.float32
    mult = mybir.AluOpType.mult
    add = mybir.AluOpType.add

    nc = bacc.Bacc("TRN2", target_bir_lowering=False, debug=False,
                   num_devices=NCORES)

    x_d = nc.dram_tensor("x", [BL, C, H, W], bf16, kind="ExternalInput").ap()
    # w1[(a, ct), :, :]: [6, 128, 128] with w1[a*2+ct, p, r] =
    #   f3[ct*128+p, r] * f1[a, r] * f2[0, r]
    w1_d = nc.dram_tensor("w1", [FH * 2, 128, R], bf16,
                          kind="ExternalInput").ap()
    f0t_d = nc.dram_tensor("f0t", [R, F], bf16, kind="ExternalInput").ap()
    # rs[r] = [f1[1]/f1[0], f1[2]/f1[0], f2[1]/f2[0], f2[2]/f2[0]]
    rs_d = nc.dram_tensor("rs", [R, 4], f32, kind="ExternalInput").ap()
    out_d = nc.dram_tensor("out", [BL, 2, 128, HP * WP], bf16,
                           kind="ExternalOutput").ap()

    with tile.TileContext(nc, trace_sim=False) as tc:
        with tc.tile_pool(name="wp", bufs=1) as wp, \
             tc.tile_pool(name="xp", bufs=2) as xp, \
             tc.tile_pool(name="t0p", bufs=3) as t0p, \
             tc.tile_pool(name="thp", bufs=2) as thp, \
             tc.tile_pool(name="yhp", bufs=3) as yhp, \
             tc.tile_pool(name="twp", bufs=2) as twp, \
             tc.tile_pool(name="zp", bufs=3) as zp, \
             tc.tile_pool(name="op", bufs=3) as op, \
             tc.tile_pool(name="ups", bufs=2, space="PSUM") as upsp, \
             tc.tile_pool(name="ops", bufs=1, space="PSUM") as opsp:

            # --- weights (resident) ---
            w1_t = wp.tile([128, FH * 2, 128], bf16)  # [c_sub, a*2+ct, r]
            nc.scalar.dma_start(w1_t[:, :, :],
                                w1_d.rearrange("k p r -> p k r"))
            rs_t = wp.tile([128, 4], f32)
            nc.scalar.dma_start(rs_t[:, :], rs_d)
            f0t_t = wp.tile([128, F], bf16)
            nc.scalar.dma_start(f0t_t[:, :], f0t_d)

            g = 0  # global band counter for flavor cycling
            for img in range(BL):
                # --- load x image: [c_sub, ct, h*w] ---
                x_t = xp.tile([128, 2, H * W], bf16, tag="x")
                qparts = ([(0, 8), (8, 8)] if img == 0 else [(0, 16)]) \
                    + [(16 * q, 16) for q in range(1, 8)]
                for (row0, nrow) in qparts:
                    for ct in range(2):
                        nc.sync.dma_start(
                            x_t[:, ct, row0 * 128:(row0 + nrow) * 128],
                            x_d[img, ct * 128:(ct + 1) * 128,
                                row0:row0 + nrow, :],
                        )

                for bi in range(NB):
                    h0 = bi * BH
                    fold, w_pool, b_dve = FLAVORS[g % len(FLAVORS)]
                    g += 1

                    u_ps = upsp.tile([128, 1024], f32, tag="u")
                    if fold:
                        # --- A (folded): 3 taps x 2 ct, 768 cols ---
                        for a in range(FH):
                            for ct in range(2):
                                for (c0, n) in ((0, 512), (512, 256)):
                                    nc.tensor.matmul(
                                        u_ps[:, c0:c0 + n],
                                        w1_t[:, a * 2 + ct, :],
                                        x_t[:, ct, (h0 + a) * 128 + c0:
                                            (h0 + a) * 128 + c0 + n],
                                        start=(a == 0 and ct == 0),
                                        stop=(a == FH - 1 and ct == 1),
                                    )
                        # yh = copy(u_f)  (ACT, PSUM->SBUF bf16)
                        yh_t = yhp.tile([128, N1], bf16, tag="yh")
                        nc.scalar.copy(yh_t[:, :], u_ps[:, 0:N1])
                    else:
                        # --- A (plain 1x1): 8 rows, tap0-scaled weights ---
                        for ct in range(2):
                            for c0 in (0, 512):
                                nc.tensor.matmul(
                                    u_ps[:, c0:c0 + 512],
                                    w1_t[:, ct, :],
                                    x_t[:, ct, h0 * 128 + c0:
                                        h0 * 128 + c0 + 512],
                                    start=(ct == 0),
                                    stop=(ct == 1),
                                )
                        t0_t = t0p.tile([128, 1024], bf16, tag="t0")
                        nc.scalar.copy(t0_t[:, :], u_ps[:, :])
                        # h-taps: TS(4x) + TT(2x), 128-elem shifts stay
                        # 4B-aligned
                        th_t = thp.tile([128, N1], bf16, tag="th")
                        yh_t = yhp.tile([128, N1], bf16, tag="yh")
                        nc.vector.tensor_scalar(
                            th_t[:, :], t0_t[:, 128:128 + N1],
                            rs_t[:, 0:1], None, op0=mult)
                        nc.vector.tensor_tensor(
                            yh_t[:, :], th_t[:, :], t0_t[:, 0:N1], op=add)
                        nc.vector.tensor_scalar(
                            th_t[:, :], t0_t[:, 256:256 + N1],
                            rs_t[:, 1:2], None, op0=mult)
                        nc.vector.tensor_tensor(
                            yh_t[:, :], yh_t[:, :], th_t[:, :], op=add)

                    # --- w-taps ---
                    z_t = zp.tile([128, NZ], bf16, tag="z")
                    zv = z_t.rearrange("p (h w) -> p h w", w=WP)
                    yv = yh_t.rearrange("p (h w) -> p h w", w=128)
                    t2_t = twp.tile([128, N1], bf16, tag="tw")
                    nc.vector.tensor_scalar(
                        t2_t[:, :], yh_t[:, :], rs_t[:, 3:4], None, op0=mult)
                    t2v = t2_t.rearrange("p (h w) -> p h w", w=128)
                    if w_pool:
                        t1_t = twp.tile([128, N1], bf16, tag="tw1")
                        nc.vector.tensor_scalar(
                            t1_t[:, :], yh_t[:, :], rs_t[:, 2:3], None,
                            op0=mult)
                        t1v = t1_t.rearrange("p (h w) -> p h w", w=128)
                        nc.gpsimd.tensor_tensor(
                            zv[:, :, :], yv[:, :, 0:WP], t1v[:, :, 1:1 + WP],
                            op=add)
                        nc.gpsimd.tensor_tensor(
                            zv[:, :, :], zv[:, :, :], t2v[:, :, 2:2 + WP],
                            op=add)
                    else:
                        # TT even-shift (2x), then STT odd tap (1x)
                        nc.vector.tensor_tensor(
                            zv[:, :, :], yv[:, :, 0:WP], t2v[:, :, 2:2 + WP],
                            op=add)
                        nc.vector.scalar_tensor_tensor(
                            zv[:, :, :], yv[:, :, 1:1 + WP], rs_t[:, 2:3],
                            zv[:, :, :], op0=mult, op1=add)

                    # --- stage B: 4 matmuls into one 3-bank PSUM tile ---
                    o_ps = opsp.tile([128, 2 * NZ], f32, tag="ops")
                    for ft in range(2):
                        for (c0, n) in ((0, 504), (504, 252)):
                            nc.tensor.matmul(
                                o_ps[:, ft * NZ + c0: ft * NZ + c0 + n],
                                f0t_t[:, ft * 128:(ft + 1) * 128],
                                z_t[:, c0:c0 + n],
                                start=True, stop=True,
                            )
                    o_t = op.tile([128, 2, NZ], bf16, tag="o")
                    if b_dve:
                        nc.vector.tensor_copy(
                            o_t.rearrange("p f n -> p (f n)"), o_ps[:, :])
                    else:
                        nc.scalar.copy(
                            o_t.rearrange("p f n -> p (f n)"), o_ps[:, :])

                    # --- store band ---
                    nc.sync.dma_start(
                        out_d[img, :, :, h0 * WP:(h0 + BH) * WP].rearrange(
                            "f p n -> p f n"),
                        o_t[:, :, :],
                    )

    nc.compile()
    return nc


def _get_nc():
    if "nc" not in _NC_CACHE:
        _NC_CACHE["nc"] = _build_nc()
    return _NC_CACHE["nc"]


def _prep_in_maps(x, f0, f1, f2, f3):
    import ml_dtypes
    bf16 = ml_dtypes.bfloat16

    f1 = np.asarray(f1, np.float64)
    f2 = np.asarray(f2, np.float64)
    h0 = f1[0].copy()
    h0[np.abs(h0) < 1e-30] = 1e-30
    w0 = f2[0].copy()
    w0[np.abs(w0) < 1e-30] = 1e-30
    # w1[a*2+ct, p, r] = f3[ct*128+p, r] * f1[a, r] * f2[0, r]
    f3w = np.asarray(f3, np.float64).reshape(2, 128, R)  # [ct, p, r]
    w1 = np.empty((FH * 2, 128, R), np.float64)
    for a in range(FH):
        for ct in range(2):
            w1[a * 2 + ct] = f3w[ct] * (f1[a] * w0)[None, :]
    w1b = np.ascontiguousarray(w1.astype(bf16))
    rs = np.stack([f1[1] / h0, f1[2] / h0, f2[1] / w0, f2[2] / w0],
                  axis=1).astype(np.float32)
    rs = np.ascontiguousarray(rs)
    f0t = np.ascontiguousarray(np.asarray(f0, np.float32).T.astype(bf16))
    xb = np.ascontiguousarray(np.asarray(x).astype(bf16))
    return [
        {"x": xb[i * BL:(i + 1) * BL], "w1": w1b, "f0t": f0t, "rs": rs}
        for i in range(NCORES)
    ]


def kernel(x, f0, f1, f2, f3):
    from concourse import bass_utils

    nc = _get_nc()
    in_maps = _prep_in_maps(x, f0, f1, f2, f3)
    res = bass_utils.run_bass_kernel_spmd(
        nc, in_maps, core_ids=list(range(NCORES)))
    # out shards are [BL, 2, 128, HP*WP]; (ft, p) merges to F contiguously.
    # bf16 -> fp32 via bit shift (exact, much faster than ml_dtypes astype).
    shards = [np.asarray(r["out"]).view(np.uint16) for r in res.results]
    raw = np.stack(shards)  # [NCORES, BL, 2, 128, HP*WP] uint16
    out = (raw.astype(np.uint32) << 16).view(np.float32)
    return np.ascontiguousarray(out.reshape(B, F, HP, WP))


# revision 6
# speedup vs baseline: 1.3225x; 1.3225x over previous
"""CP-decomposed conv (1x1 -> depthwise-h -> depthwise-w -> 1x1) on 8 TRN2
NeuronCores, data-parallel over batch (4 images per core).

Per-core pipeline:
  stage A: u[r, h', w] = sum_{a,c} x[c, h'+a, w] * W1[(a,c), r]
           (h-depthwise folded into the channel-mixing matmul via
            host-precomputed W1 = f1 (x) f3; bf16 matmuls, fp32 PSUM accum)
  w-taps:  z[r, h', w'] = sum_cw u[r, h', w'+cw] * f2[cw, r], computed as
             ze = u * f2[0]            (fused into PSUM->SBUF copy, ACT)
             z  = ze[+1]*r1 + ze[+0]   (DVE)    r_c = f2[cw]/f2[0]
             z  = ze[+2]*r2 + z        (DVE)
  stage B: out[f, h', w'] = sum_r f0[f, r] * z[r, h', w']  (bf16 matmul)
           into one 2-bank PSUM tile per ftile (matmul dst may cross PSUM
           bank boundaries), single wide PSUM->SBUF bf16 copy per ftile.
Output is stored bf16 in a DMA-friendly [img, ftile, p, h'*w'] layout and
reshaped/upcast on host.

LDWEIGHTS dedup: walrus' --enable-ldw-opt is force-enabled via a monkeypatch
(concourse hardcodes it off); correctness is validated by the test harness.
"""

import numpy as np

B, C, H, W = 32, 256, 128, 128
FH, FW = 3, 3
F, R = 256, 128
HP, WP = H - FH + 1, W - FW + 1  # 126, 126
NCORES = 8
BL = B // NCORES  # images per core

# 126 output rows: short first band (fast pipeline ramp), 15x8, short tail.
BANDS = [(0, 4)] + [(4 + i * 8, 8) for i in range(15)] + [(124, 2)]

# walrus' visitInstLdweights crashes with --enable-ldw-opt=true; keep off.
ENABLE_LDW_OPT = False


def _chunks(bh):
    out = []
    r0 = 0
    while r0 < bh:
        nr = min(4, bh - r0)
        out.append((r0, nr))
        r0 += nr
    return out


_NC_CACHE = {}


def _patch_ldw_opt():
    """Flip walrus' --enable-ldw-opt to true (concourse hardcodes false)."""
    if not ENABLE_LDW_OPT:
        return
    from concourse import bass_utils as bu
    fn = bu.bir_verify_and_optimise
    if getattr(fn, "_ldw_patched", False):
        return

    inner = fn
    def patched(*args, **kwargs):
        import concourse.bass_utils as _bu
        orig_run = _bu.run_command

        def run_patched(cmd, **kw):
            cmd = ["--enable-ldw-opt=true" if c == "--enable-ldw-opt=false"
                   else c for c in cmd]
            return orig_run(cmd, **kw)

        _bu.run_command = run_patched
        try:
            return inner(*args, **kwargs)
        finally:
            _bu.run_command = orig_run

    patched._ldw_patched = True
    bu.bir_verify_and_optimise = patched


def _build_nc():
    import concourse.bacc as bacc
    import concourse.mybir as mybir
    import concourse.tile as tile

    dt = mybir.dt
    bf16 = dt.bfloat16
    f32 = dt.float32
    mult = mybir.AluOpType.mult
    add = mybir.AluOpType.add

    _patch_ldw_opt()
    nc = bacc.Bacc("TRN2", target_bir_lowering=False, debug=False,
                   num_devices=NCORES)

    x_d = nc.dram_tensor("x", [BL, C, H, W], bf16, kind="ExternalInput").ap()
    w1_d = nc.dram_tensor("w1", [FH * C, R], bf16, kind="ExternalInput").ap()
    f0t_d = nc.dram_tensor("f0t", [R, F], bf16, kind="ExternalInput").ap()
    # f2s[r] = [f2[0,r], f2[1,r]/f2[0,r], f2[2,r]/f2[0,r]]
    f2s_d = nc.dram_tensor("f2s", [R, FW], f32, kind="ExternalInput").ap()
    out_d = nc.dram_tensor("out", [BL, 2, 128, HP * WP], bf16,
                           kind="ExternalOutput").ap()

    with tile.TileContext(nc, trace_sim=False) as tc:
        with tc.tile_pool(name="wp", bufs=1) as wp, \
             tc.tile_pool(name="xp", bufs=2) as xp, \
             tc.tile_pool(name="ep", bufs=4) as ep, \
             tc.tile_pool(name="zp", bufs=4) as zp, \
             tc.tile_pool(name="op", bufs=4) as op, \
             tc.tile_pool(name="ups", bufs=2, space="PSUM") as upsp, \
             tc.tile_pool(name="ops", bufs=2, space="PSUM") as opsp:

            # --- weights (resident) ---
            w1_t = wp.tile([128, FH * 2, 128], bf16)  # [c_sub, kt=a*2+ct, r]
            nc.scalar.dma_start(
                w1_t[:, :, :],
                w1_d.rearrange("(kt p) r -> p kt r", p=128),
            )
            f2s_t = wp.tile([128, FW], f32)
            nc.scalar.dma_start(f2s_t[:, :], f2s_d)
            f0t_t = wp.tile([128, F], bf16)
            nc.scalar.dma_start(f0t_t[:, :], f0t_d)

            ci = 0
            for img in range(BL):
                # --- load x image: [c_sub, ct, h*w] ---
                x_t = xp.tile([128, 2, H * W], bf16, tag="x")
                qparts = ([(0, 8), (8, 8)] if img == 0 else [(0, 16)]) \
                    + [(16 * q, 16) for q in range(1, 8)]
                for (row0, nrow) in qparts:
                    for ct in range(2):
                        nc.sync.dma_start(
                            x_t[:, ct, row0 * 128:(row0 + nrow) * 128],
                            x_d[img, ct * 128:(ct + 1) * 128,
                                row0:row0 + nrow, :],
                        )

                for (h0, bh) in BANDS:
                    # --- stage A: u in PSUM [r, bh*128]; weight-outer so
                    # the stationary tile stays put across chunks ---
                    u_ps = upsp.tile([128, 8 * 128], f32, tag="u")
                    for a in range(FH):
                        for ct in range(2):
                            for (r0, nr) in _chunks(bh):
                                n = nr * 128
                                row = h0 + r0 + a
                                nc.tensor.matmul(
                                    u_ps[:, r0 * 128: r0 * 128 + n],
                                    w1_t[:, a * 2 + ct, :],
                                    x_t[:, ct, row * 128: row * 128 + n],
                                    start=(a == 0 and ct == 0),
                                    stop=(a == FH - 1 and ct == 1),
                                )

                    # --- fused PSUM->SBUF copy * f2[0] (tap0), then taps ---
                    ze_t = ep.tile([128, 8 * 128], bf16, tag="ze")
                    nc.scalar.mul(ze_t[:, 0:bh * 128], u_ps[:, 0:bh * 128],
                                  f2s_t[:, 0:1])
                    z_t = zp.tile([128, 8 * WP], bf16, tag="z")
                    zv = z_t[:, 0:bh * WP].rearrange("p (h w) -> p h w", w=WP)
                    zev = ze_t[:, 0:bh * 128].rearrange(
                        "p (h w) -> p h w", w=128)
                    nc.vector.scalar_tensor_tensor(
                        zv, zev[:, :, 1:1 + WP], f2s_t[:, 1:2],
                        zev[:, :, 0:WP], op0=mult, op1=add)
                    nc.vector.scalar_tensor_tensor(
                        zv, zev[:, :, 2:2 + WP], f2s_t[:, 2:3],
                        zv, op0=mult, op1=add)

                    # --- stage B (bf16): one 2-bank PSUM tile per ftile,
                    # 504-col matmuls (dst may cross the bank boundary),
                    # single wide PSUM->SBUF copy ---
                    o_t = op.tile([128, 2, 8 * WP], bf16, tag="o")
                    for ft in range(2):
                        for (r0, nr) in _chunks(bh):
                            n = nr * WP
                            o_ps = opsp.tile([128, 504], f32, tag="ops")
                            nc.tensor.matmul(
                                o_ps[:, 0:n],
                                f0t_t[:, ft * 128:(ft + 1) * 128],
                                z_t[:, r0 * WP: r0 * WP + n],
                                start=True, stop=True,
                            )
                            dst = o_t[:, ft, r0 * WP: r0 * WP + n]
                            if ci % 4 == 3:
                                nc.vector.tensor_copy(dst, o_ps[:, 0:n])
                            else:
                                nc.scalar.copy(dst, o_ps[:, 0:n])
                            ci += 1

                    # --- store band (contiguous per partition) ---
                    nc.sync.dma_start(
                        out_d[img, :, :, h0 * WP:(h0 + bh) * WP].rearrange(
                            "f p n -> p f n"),
                        o_t[:, :, 0: bh * WP],
                    )

    nc.compile()
    return nc


def _get_nc():
    if "nc" not in _NC_CACHE:
        _NC_CACHE["nc"] = _build_nc()
    return _NC_CACHE["nc"]


def _prep_in_maps(x, f0, f1, f2, f3):
    import ml_dtypes
    bf16 = ml_dtypes.bfloat16

    # W1[(a, c), r] = f1[a, r] * f3[c, r]
    w1 = (np.asarray(f1, np.float32)[:, None, :]
          * np.asarray(f3, np.float32)[None, :, :]).reshape(FH * C, R)
    w1b = np.ascontiguousarray(w1.astype(bf16))
    f0t = np.ascontiguousarray(np.asarray(f0, np.float32).T.astype(bf16))
    f2 = np.asarray(f2, np.float64)
    s0 = f2[0].copy()
    s0[np.abs(s0) < 1e-30] = 1e-30
    f2s = np.stack([s0, f2[1] / s0, f2[2] / s0], axis=1).astype(np.float32)
    f2s = np.ascontiguousarray(f2s)
    xb = np.ascontiguousarray(np.asarray(x).astype(bf16))
    return [
        {"x": xb[i * BL:(i + 1) * BL], "w1": w1b, "f0t": f0t, "f2s": f2s}
        for i in range(NCORES)
    ]


def kernel(x, f0, f1, f2, f3):
    from concourse import bass_utils

    nc = _get_nc()
    in_maps = _prep_in_maps(x, f0, f1, f2, f3)
    res = bass_utils.run_bass_kernel_spmd(
        nc, in_maps, core_ids=list(range(NCORES)))
    # out shards are [BL, 2, 128, HP*WP]; (ft, p) merges to F contiguously.
    # bf16 -> fp32 via bit shift (exact, much faster than ml_dtypes astype).
    shards = [np.asarray(r["out"]).view(np.uint16) for r in res.results]
    raw = np.stack(shards)  # [NCORES, BL, 2, 128, HP*WP] uint16
    out = (raw.astype(np.uint32) << 16).view(np.float32)
    return np.ascontiguousarray(out.reshape(B, F, HP, WP))
